# revision 1
# baseline (speedup 1.0000x reference)
"""AttentionQV0 TRN2 kernel: builder + host prep + gather.

Per-core work: one (b,p) and 4 heads. 8 cores = 2*2 bp x 2 head-groups.
All matmul operands fp16 (1 cyc/row on PE; ~5e-4 quantization), f32 PSUM
accumulation, f32 vector/scalar math. Device dataflow per head:
  QK (psum, free enum (w2,h2)) -> v.transpose -> A (c-layout, free (w2,w1))
  -> ACT scale pass (*u0*v0 per channel, fp16) -> dw-convs as two stt taps
     (ratio-factored, GPSIMD + DVE) -> a3 fp16
  -> 1x1 matmul (host-permuted plain W, rhs enum (w1,w2)) -> Y psum
  -> ACT BN+SiLU -> Ys -> v.transpose -> S2T (rows m) -> ACT exp -> E fp16
  -> AV matmul (lhsT = v_aug incl ones col) -> AVt psum [65, n]
  -> rs row -> ln/exp recip -> K=1 bcast matmul -> TT normalize -> avn fp16
  -> out-proj partial accumulated over heads -> DRAM.
Host folds ln_g/ln_b/DH^-0.5 into qkv weights, permutes w_1x1 for the
layout bijections, sums core pairs + b_out at gather.
"""

import numpy as np
import concourse.bass as bass
import concourse.bacc as bacc
import concourse.mybir as mybir
import concourse.tile as tile
from concourse.tile import add_dep_helper

dt = mybir.dt
AF = mybir.ActivationFunctionType
ALU = mybir.AluOpType

B, P, N, DIM = 2, 2, 1024, 256
HEADS, DH, HW, C = 8, 64, 32, 1024
NT = N // 128  # 8 n-tiles
LN_EPS, BN_EPS = 1e-5, 1e-5
NP_F16 = np.float16

# ---------------------------------------------------------------- host prep

_TT, _PP = np.meshgrid(np.arange(8), np.arange(128), indexing="ij")
CMAP = (4 * _TT + _PP // 32) * 32 + (_PP % 32)        # [8,128] c(t,p)
OMAP = (_PP % 32) * 32 + (4 * _TT + _PP // 32)        # [8,128] o(u',p')


def host_prep(core: int, inputs: dict) -> dict:
    f32 = np.float32
    bp = core // 2
    b, p_ = bp // 2, bp % 2
    heads = [4 * (core % 2) + i for i in range(4)]

    x = np.ascontiguousarray(inputs["x"][b, p_], dtype=f32)
    ln_g = inputs["ln_g"].astype(f32)
    ln_b = inputs["ln_b"].astype(f32)
    w_qkv = inputs["w_qkv"].astype(f32)
    wc1 = inputs["w_conv1"].astype(f32)
    wc2 = inputs["w_conv2"].astype(f32)
    w1x1 = inputs["w_1x1"].astype(f32)
    bn_g, bn_b = inputs["bn_g"].astype(f32), inputs["bn_b"].astype(f32)
    bn_m, bn_v = inputs["bn_m"].astype(f32), inputs["bn_v"].astype(f32)
    w_out = inputs["w_out"].astype(f32)

    Wg = w_qkv * ln_g[:, None]
    rb = (ln_b @ w_qkv).astype(f32)
    sc = np.float32(DH ** -0.5)

    def cols(base, scale):
        cc = np.concatenate([Wg[:, base + h * DH: base + (h + 1) * DH] for h in heads], axis=1) * scale
        rr = np.concatenate([rb[base + h * DH: base + (h + 1) * DH] for h in heads]) * scale
        return np.concatenate([cc, rr[None, :]], axis=0)  # [257, 256]

    qa = cols(0, sc)
    ka = cols(512, np.float32(1.0))
    va = cols(1024, np.float32(1.0))
    vw = np.zeros((257, 4 * 65), f32)
    for i in range(4):
        vw[:, i * 65: i * 65 + 64] = va[:, i * 64:(i + 1) * 64]
        vw[256, i * 65 + 64] = 1.0
    wqkv = np.concatenate([qa, ka, vw], axis=1).astype(NP_F16)  # [257, 772]

    eps = np.float32(1e-30)
    u0 = np.where(np.abs(wc1[:, 0]) < eps, eps, wc1[:, 0])
    v0 = np.where(np.abs(wc2[:, 0]) < eps, eps, wc2[:, 0])
    r1 = (wc1[:, 1] / u0).astype(f32)
    r2 = (wc2[:, 1] / v0).astype(f32)
    uv = (u0 * v0).astype(f32)
    conv_r = np.zeros((128, 24), f32)
    conv_r[:, 0:8] = r1[CMAP].T
    conv_r[:, 8:16] = r2[CMAP].T
    conv_r[:, 16:24] = uv[CMAP].T

    w1x1T = np.zeros((128, 8, 1024), NP_F16)
    for t in range(8):
        wt = w1x1[:, CMAP[t]]  # [1024(o), 128(c rows of tile t)]
        for u in range(8):
            w1x1T[:, t, u * 128:(u + 1) * 128] = wt[OMAP[u], :].T.astype(NP_F16)

    inv = 1.0 / np.sqrt(bn_v + BN_EPS)
    bnsc = (inv * bn_g).astype(f32)
    bnbi = (bn_b - bn_m * bnsc).astype(f32)
    bn_sb = np.zeros((128, 16), f32)
    bn_sb[:, 0:8] = bnsc[OMAP].T
    bn_sb[:, 8:16] = bnbi[OMAP].T

    wout = np.concatenate([w_out[h * DH:(h + 1) * DH, :] for h in heads], axis=0).astype(NP_F16)

    ident = np.eye(128, dtype=f32)
    ones_row = np.ones((1, 1024), NP_F16)

    return {
        "x_bp": x,                       # [1024, 256] f32
        "wqkv_a": np.ascontiguousarray(wqkv[0:128]),
        "wqkv_b": np.ascontiguousarray(wqkv[128:256]),
        "wqkv_c": np.ascontiguousarray(wqkv[256:257]),
        "w1x1t": np.ascontiguousarray(w1x1T.reshape(128, 8192)),
        "conv_r": conv_r,
        "bn_sb": bn_sb,
        "wout": wout,
        "ident": ident,
        "ones_row": ones_row,
    }


def gather(parts, b_out):
    out = np.zeros((B, P, N, DIM), np.float32)
    for bp in range(4):
        out[bp // 2, bp % 2] = parts[2 * bp] + parts[2 * bp + 1] + b_out[None, :].astype(np.float32)
    return out


# ---------------------------------------------------------------- device

def declare_tensors(nc):
    f16, f32 = dt.float16, dt.float32
    t = {}
    def mk(name, shape, dty, kind="ExternalInput"):
        t[name] = nc.dram_tensor(name, shape, dty, kind=kind).ap()
    mk("x_bp", [1024, 256], f32)
    mk("wqkv_a", [128, 772], f16)
    mk("wqkv_b", [128, 772], f16)
    mk("wqkv_c", [1, 772], f16)
    mk("w1x1t", [128, 8192], f16)
    mk("conv_r", [128, 24], f32)
    mk("bn_sb", [128, 16], f32)
    mk("wout", [256, 256], f16)
    mk("ident", [128, 128], f32)
    mk("ones_row", [1, 1024], f16)
    mk("out_part", [1024, 256], f32, kind="ExternalOutput")
    return t


def en_wh(ap1024, lo, hi):
    """Enumerate a 1024-wide free range as (outer idx in [lo,hi), inner 32) with
    addr = inner*32 + outer."""
    r = ap1024.rearrange("p (a b) -> p a b", a=32)   # a = addr//32, b = addr%32
    return r.transpose([0, 2, 1])[:, lo:hi, :]


def emit(tc, dr, sim_safe=False, gp_conv=False):
    nc = tc.nc
    f32, f16 = dt.float32, dt.float16

    from contextlib import ExitStack
    stack = ExitStack()
    const = stack.enter_context(tc.tile_pool(name="const", bufs=1))
    xglob = const.tile([128, 8, 256], f32, tag="xglob")
    nc.sync.dma_start(xglob[:], dr["x_bp"].rearrange("(t p) d -> p t d", p=128))
    conv_r = const.tile([128, 24], f32, tag="conv_r")
    nc.sync.dma_start(conv_r[:], dr["conv_r"])
    bn_sb = const.tile([128, 16], f32, tag="bn_sb")
    nc.sync.dma_start(bn_sb[:], dr["bn_sb"])
    wout = const.tile([64, 4, 256], f16, tag="wout")
    for hi in range(4):
        nc.sync.dma_start(wout[:, hi, :], dr["wout"][hi * 64:(hi + 1) * 64])
    ones_row = const.tile([1, 1024], f16, tag="ones_row")
    nc.sync.dma_start(ones_row[:], dr["ones_row"])
    w1x1t = const.tile([128, 8192], f16, tag="w1x1t")

    q_sb = const.tile([64, 4, 1024], f16, tag="q_sb")
    k_sb = const.tile([64, 4, 1024], f16, tag="k_sb")
    v_sb = const.tile([128, 8, 260], f16, tag="v_sb")
    xnt = const.tile([128, 2, 1024], f16, tag="xnt")
    avn = const.tile([64, 4, 1024], f16, tag="avn")
    out_sb = const.tile([128, 8, 256], f32, tag="out_sb")

    # ---------------- startup: LN + xnT + qkv ----------------
    with tc.tile_pool(name="startup", bufs=1) as su, \
         tc.tile_pool(name="ps_init", bufs=2, space="PSUM") as psi:
        ident = su.tile([128, 128], f32, tag="ident")
        nc.sync.dma_start(ident[:], dr["ident"])
        wqa = su.tile([128, 772], f16, tag="wqa")
        nc.sync.dma_start(wqa[:], dr["wqkv_a"])
        wqb = su.tile([128, 772], f16, tag="wqb")
        nc.sync.dma_start(wqb[:], dr["wqkv_b"])
        wqc = su.tile([1, 772], f16, tag="wqc")
        nc.sync.dma_start(wqc[:], dr["wqkv_c"])

        x = xglob
        nc.sync.dma_start(w1x1t[:], dr["w1x1t"])
        s1 = su.tile([128, 8], f32, tag="s1")
        s2 = su.tile([128, 8], f32, tag="s2")
        scr = su.tile([128, 256], f32, tag="scr")
        mu8 = su.tile([128, 8], f32, tag="mu8")
        var8 = su.tile([128, 8], f32, tag="var8")
        rstd8 = su.tile([128, 8], f32, tag="rstd8")
        t8 = su.tile([128, 8], f32, tag="t8")
        xn = su.tile([128, 8, 256], f32, tag="xn")
        for j in range(NT):
            nc.vector.tensor_reduce(s1[:, j:j+1], x[:, j, :], mybir.AxisListType.X, ALU.add)
            nc.scalar.activation(scr[:], x[:, j, :], AF.Square, accum_out=s2[:, j:j+1])
        nc.vector.tensor_scalar(mu8[:], s1[:], 1.0 / 256, None, ALU.mult)
        nc.vector.tensor_tensor(t8[:], mu8[:], mu8[:], op=ALU.mult)
        nc.vector.tensor_scalar(t8[:], t8[:], -1.0, LN_EPS, ALU.mult, ALU.add)
        nc.vector.scalar_tensor_tensor(var8[:], s2[:], 1.0 / 256, t8[:], op0=ALU.mult, op1=ALU.add)
        nc.scalar.activation(t8[:], var8[:], AF.Ln)
        nc.scalar.activation(rstd8[:], t8[:], AF.Exp, scale=-0.5)
        for j in range(NT):
            nc.vector.tensor_scalar(xn[:, j, :], x[:, j, :], mu8[:, j:j+1], rstd8[:, j:j+1],
                                    ALU.subtract, ALU.mult)
        for j in range(NT):
            for half in range(2):
                pt = psi.tile([128, 128], f32, tag="pt")
                nc.tensor.transpose(pt[:], xn[:, j, half * 128:(half + 1) * 128], ident[:])
                nc.scalar.activation(xnt[:, half, j * 128:(j + 1) * 128], pt[:], AF.Copy)

        for qk in range(2):  # 0=q, 1=k
            for h in range(4):
                base = qk * 256
                lhs_a = wqa[:, base + h * 64: base + (h + 1) * 64]
                lhs_b = wqb[:, base + h * 64: base + (h + 1) * 64]
                lhs_c = wqc[:, base + h * 64: base + (h + 1) * 64]
                ps_qk = psi.tile([64, 1024], f32, tag="ps_qk")
                for ch in range(2):
                    sl = slice(ch * 512, (ch + 1) * 512)
                    nc.tensor.matmul(ps_qk[:, sl], lhs_a, xnt[:, 0, sl], start=True, stop=False)
                    nc.tensor.matmul(ps_qk[:, sl], lhs_b, xnt[:, 1, sl], start=False, stop=False)
                    nc.tensor.matmul(ps_qk[:, sl], lhs_c, ones_row[:, sl], start=False, stop=True)
                if qk == 0:
                    nc.scalar.activation(q_sb[:, h, :], ps_qk[:], AF.Copy)
                else:
                    # read-permute psum (h2,w2) in (w2,h2) order; write k_sb contiguous
                    kin = ps_qk[:].rearrange("p (h2 w2) -> p h2 w2", h2=32).transpose([0, 2, 1])
                    nc.scalar.activation(k_sb[:, h, :], kin, AF.Copy)
        for r in range(NT):
            ps_v = psi.tile([128, 260], f32, tag="ps_v")
            nsl = slice(r * 128, (r + 1) * 128)
            nc.tensor.matmul(ps_v[:], xnt[:, 0, nsl], wqa[:, 512:772], start=True, stop=False)
            nc.tensor.matmul(ps_v[:], xnt[:, 1, nsl], wqb[:, 512:772], start=False, stop=False)
            nc.tensor.matmul(ps_v[:], ones_row[:, nsl], wqc[:, 512:772], start=False, stop=True)
            nc.scalar.activation(v_sb[:, r, :], ps_v[:], AF.Copy)

    # odd heads' q/k moved to partitions 64..127 for QK row-group packing
    qk_hi = const.tile([128, 2, 2048], f16, tag="qk_hi")
    for p2 in range(2):
        nc.sync.dma_start(qk_hi[64:128, p2, 0:1024], q_sb[0:64, 2 * p2 + 1, :])
        nc.sync.dma_start(qk_hi[64:128, p2, 1024:2048], k_sb[0:64, 2 * p2 + 1, :])

    # ---------------- per-head pipeline ----------------
    work = stack.enter_context(tc.tile_pool(name="work", bufs=1))
    dram = stack.enter_context(tc.tile_pool(name="dram", bufs=1, space="DRAM"))
    ps = stack.enter_context(tc.tile_pool(name="ps", bufs=4, space="PSUM"))

    def conv_chain(ps_s, a3, j):
        at = work.tile([128, 1024], f32, tag="at", bufs=3)
        nc.vector.transpose(at[:], ps_s[:])
        ap16 = work.tile([128, 1024], f16, tag="ap16", bufs=3)
        nc.scalar.activation(ap16[:], at[:], AF.Copy, scale=conv_r[:, 16 + j:17 + j])
        r1s = conv_r[:, j:j+1]
        r2s = conv_r[:, 8+j:9+j]
        a1 = ap16[:].rearrange("p (w2 w1) -> p w2 w1", w2=32)
        t1 = work.tile([128, 1024], f16, tag="t1", bufs=2)
        t1r = t1[:].rearrange("p (w2 w1) -> p w2 w1", w2=32)
        nc.vector.scalar_tensor_tensor(t1r[:, 0:31, :], a1[:, 1:32, :], r1s, a1[:, 0:31, :],
                                       op0=ALU.mult, op1=ALU.add)
        nc.vector.scalar_tensor_tensor(t1r[:, 31, :], a1[:, 30, :], r1s, a1[:, 31, :],
                                       op0=ALU.mult, op1=ALU.add)
        a3n = a3[:, j, :].rearrange("p (w1 w2) -> p w1 w2", w1=32)
        t1p = t1[:].rearrange("p (w2 w1) -> p w2 w1", w2=32).transpose([0, 2, 1])  # [p, w1, w2]
        nc.vector.scalar_tensor_tensor(a3n[:, 0:31, :], t1p[:, 1:32, :], r2s, t1p[:, 0:31, :],
                                       op0=ALU.mult, op1=ALU.add)
        nc.vector.scalar_tensor_tensor(a3n[:, 31, :], t1p[:, 30, :], r2s, t1p[:, 31, :],
                                       op0=ALU.mult, op1=ALU.add)

    def qk_pair(p2):
        he, ho = 2 * p2, 2 * p2 + 1
        a3e = work.tile([128, 8, 1024], f16, tag="a3", bufs=2, name=f"a3e{p2}")
        a3o = work.tile([128, 8, 1024], f16, tag="a3", bufs=2, name=f"a3o{p2}")
        for j in range(NT):
            ps_se = ps.tile([128, 1024], f32, tag="ps", bufs=4, name=f"pse{p2}_{j}")
            ps_so = ps.tile([128, 1024], f32, tag="ps", bufs=4, name=f"pso{p2}_{j}")
            for ch in range(2):
                sl = slice(ch * 512, (ch + 1) * 512)
                nc.tensor.matmul(ps_se[:, sl], q_sb[:, he, j * 128:(j + 1) * 128],
                                 k_sb[:, he, sl], start=True, stop=True)
                nc.tensor.matmul(ps_so[:, sl], qk_hi[64:128, p2, j * 128:(j + 1) * 128],
                                 qk_hi[64:128, p2, 1024 + ch * 512:1024 + (ch + 1) * 512],
                                 start=True, stop=True, tile_position=(64, 0))
            conv_chain(ps_se, a3e, j)
            conv_chain(ps_so, a3o, j)
        return {he: a3e, ho: a3o}

    a3_list = qk_pair(0)
    for hi in range(4):
        a3 = a3_list.pop(hi)
        # 1x1 conv + BN/SiLU + vtrans2
        s2t_slabs = []
        first_silu_inst = None
        last_silu_inst = None
        for u in range(8):
            ps_y = ps.tile([128, 1024], f32, tag="ps", bufs=4)
            for t in range(8):
                lhs = w1x1t[:, t * 1024 + u * 128: t * 1024 + (u + 1) * 128]
                rhs = a3[:, t, :]
                for ch in range(2):
                    nc.tensor.matmul(ps_y[:, ch * 512:(ch + 1) * 512], lhs,
                                     rhs[:, ch * 512:(ch + 1) * 512],
                                     start=(t == 0), stop=(t == 7))
            ys = work.tile([128, 1024], f32, tag="ys", bufs=3)
            if sim_safe:
                zz = work.tile([128, 1024], f32, tag="zz", bufs=1)
                nc.vector.tensor_scalar(zz[:], ps_y[:], bn_sb[:, u:u+1], bn_sb[:, 8+u:9+u],
                                        ALU.mult, ALU.add)
                sg = work.tile([128, 1024], f32, tag="sg", bufs=1)
                si = nc.scalar.activation(sg[:], ps_y[:], AF.Sigmoid, bias=bn_sb[:, 8+u:9+u],
                                          scale=bn_sb[:, u:u+1])
                nc.vector.tensor_tensor(ys[:], zz[:], sg[:], op=ALU.mult)
            else:
                si = nc.scalar.activation(ys[:], ps_y[:], AF.Silu, bias=bn_sb[:, 8+u:9+u],
                                          scale=bn_sb[:, u:u+1])
            if first_silu_inst is None:
                first_silu_inst = si
            last_silu_inst = si
            s2t_u = work.tile([128, 1024], f32, tag="s2t", bufs=8)
            nc.vector.transpose(s2t_u[:], ys[:])
            s2t_slabs.append(s2t_u)



        if hi == 1:
            a3_list.update(qk_pair(1))
        # exp + AV
        ps_av = ps.tile([128, 1024], f32, tag="ps", bufs=4)  # rows 0:65 used
        first_exp_inst = None
        for r in range(8):
            e = work.tile([128, 1024], f16, tag="e", bufs=3)
            sin = s2t_slabs[r][:].rearrange("p (w1 o1) -> p w1 o1", w1=32).transpose([0, 2, 1])
            ei = nc.scalar.activation(e[:], sin, AF.Exp)
            if first_exp_inst is None:
                first_exp_inst = ei
            lhs = v_sb[:, r, hi * 65: hi * 65 + 65]
            for ch in range(2):
                nc.tensor.matmul(ps_av[0:65, ch * 512:(ch + 1) * 512], lhs,
                                 e[:, ch * 512:(ch + 1) * 512],
                                 start=(r == 0), stop=(r == 7))

        rln = work.tile([1, 1024], f32, tag="rln")
        nc.scalar.activation(rln[:], ps_av[64:65, :], AF.Ln)
        rinv = work.tile([1, 1024], f32, tag="rinv")
        prev_rinv_inst = nc.scalar.activation(rinv[:], rln[:], AF.Exp, scale=-1.0)
        rbc = work.tile([64, 1024], f32, tag="rbc", bufs=2)
        rdram = dram.tile([1, 1024], f32, tag="rdram", bufs=2)
        nc.sync.dma_start(rdram[:], rinv[:])
        rinv_bc = bass.AP(tensor=rdram.tensor, offset=rdram[:].offset,
                          ap=[[0, 64]] + list(rdram[:].ap)[1:])
        nc.sync.dma_start(rbc[:], rinv_bc)
        nc.vector.tensor_tensor(avn[:, hi, :], ps_av[0:64, :], rbc[:], op=ALU.mult)

    # ---------------- out projection ----------------
    for j in range(NT):
        ps_o = ps.tile([128, 1024], f32, tag="ps", bufs=4)  # use [128, 0:256]
        for hi in range(4):
            nc.tensor.matmul(ps_o[:, 0:256], avn[:, hi, j * 128:(j + 1) * 128],
                             wout[:, hi, :], start=(hi == 0), stop=(hi == 3))
        nc.scalar.activation(out_sb[:, j, :], ps_o[:, 0:256], AF.Copy)
    nc.sync.dma_start(dr["out_part"].rearrange("(t p) d -> p t d", p=128), out_sb[:])

    stack.close()


# ================================================================ driver

_CACHE = {}


def _build_program():
    nc = bacc.Bacc("TRN2", target_bir_lowering=False, debug=False, num_devices=8)
    dr = declare_tensors(nc)
    with tile.TileContext(nc) as tc:
        emit(tc, dr, sim_safe=False)
    nc.compile()
    return nc


def kernel(**inputs):
    """Full-input AttentionQV0 forward on 8 TRN2 NeuronCores.

    Shards (b,p,head-group) across cores, runs the Bass kernel, gathers
    partial head-group outputs host-side (sum core pairs + b_out).
    """
    from concourse.bass_utils import run_bass_kernel_spmd
    inputs = {k: np.asarray(v) for k, v in inputs.items()}
    if "nc" not in _CACHE:
        _CACHE["nc"] = _build_program()
    nc = _CACHE["nc"]
    in_maps = [host_prep(c, inputs) for c in range(8)]
    res = run_bass_kernel_spmd(nc, in_maps, list(range(8)))
    parts = [res.results[c]["out_part"] for c in range(8)]
    return gather(parts, inputs["b_out"])


# revision 2
# speedup vs baseline: 1.0293x; 1.0293x over previous
"""AttentionQV0 TRN2 kernel: builder + host prep + gather.

Per-core work: one (b,p) and 4 heads. 8 cores = 2*2 bp x 2 head-groups.
All matmul operands fp16 (1 cyc/row on PE; ~5e-4 quantization), f32 PSUM
accumulation, f32 vector/scalar math. Device dataflow per head:
  QK (psum, free enum (w2,h2)) -> v.transpose -> A (c-layout, free (w2,w1))
  -> ACT scale pass (*u0*v0 per channel, fp16) -> dw-convs as two stt taps
     (ratio-factored, GPSIMD + DVE) -> a3 fp16
  -> 1x1 matmul (host-permuted plain W, rhs enum (w1,w2)) -> Y psum
  -> ACT BN+SiLU -> Ys -> v.transpose -> S2T (rows m) -> ACT exp -> E fp16
  -> AV matmul (lhsT = v_aug incl ones col) -> AVt psum [65, n]
  -> rs row -> ln/exp recip -> K=1 bcast matmul -> TT normalize -> avn fp16
  -> out-proj partial accumulated over heads -> DRAM.
Host folds ln_g/ln_b/DH^-0.5 into qkv weights, permutes w_1x1 for the
layout bijections, sums core pairs + b_out at gather.
"""

import numpy as np
import concourse.bass as bass
import concourse.bacc as bacc
import concourse.mybir as mybir
import concourse.tile as tile
from concourse.tile import add_dep_helper

dt = mybir.dt
AF = mybir.ActivationFunctionType
ALU = mybir.AluOpType

B, P, N, DIM = 2, 2, 1024, 256
HEADS, DH, HW, C = 8, 64, 32, 1024
NT = N // 128  # 8 n-tiles
LN_EPS, BN_EPS = 1e-5, 1e-5
NP_F16 = np.float16

# ---------------------------------------------------------------- host prep

_TT, _PP = np.meshgrid(np.arange(8), np.arange(128), indexing="ij")
CMAP = (4 * _TT + _PP // 32) * 32 + (_PP % 32)        # [8,128] c(t,p)
OMAP = (_PP % 32) * 32 + (4 * _TT + _PP // 32)        # [8,128] o(u',p')


def host_prep(core: int, inputs: dict) -> dict:
    f32 = np.float32
    bp = core // 2
    b, p_ = bp // 2, bp % 2
    heads = [4 * (core % 2) + i for i in range(4)]

    x = np.ascontiguousarray(inputs["x"][b, p_], dtype=f32)
    ln_g = inputs["ln_g"].astype(f32)
    ln_b = inputs["ln_b"].astype(f32)
    w_qkv = inputs["w_qkv"].astype(f32)
    wc1 = inputs["w_conv1"].astype(f32)
    wc2 = inputs["w_conv2"].astype(f32)
    w1x1 = inputs["w_1x1"].astype(f32)
    bn_g, bn_b = inputs["bn_g"].astype(f32), inputs["bn_b"].astype(f32)
    bn_m, bn_v = inputs["bn_m"].astype(f32), inputs["bn_v"].astype(f32)
    w_out = inputs["w_out"].astype(f32)

    Wg = w_qkv * ln_g[:, None]
    rb = (ln_b @ w_qkv).astype(f32)
    sc = np.float32(DH ** -0.5)

    def cols(base, scale):
        cc = np.concatenate([Wg[:, base + h * DH: base + (h + 1) * DH] for h in heads], axis=1) * scale
        rr = np.concatenate([rb[base + h * DH: base + (h + 1) * DH] for h in heads]) * scale
        return np.concatenate([cc, rr[None, :]], axis=0)  # [257, 256]

    qa = cols(0, sc)
    ka = cols(512, np.float32(1.0))
    va = cols(1024, np.float32(1.0))
    vw = np.zeros((257, 4 * 65), f32)
    for i in range(4):
        vw[:, i * 65: i * 65 + 64] = va[:, i * 64:(i + 1) * 64]
        vw[256, i * 65 + 64] = 1.0
    wqkv = np.concatenate([qa, ka, vw], axis=1).astype(NP_F16)  # [257, 772]

    eps = np.float32(1e-30)
    u0 = np.where(np.abs(wc1[:, 0]) < eps, eps, wc1[:, 0])
    v0 = np.where(np.abs(wc2[:, 0]) < eps, eps, wc2[:, 0])
    r1 = (wc1[:, 1] / u0).astype(f32)
    r2 = (wc2[:, 1] / v0).astype(f32)
    uv = (u0 * v0).astype(f32)
    conv_r = np.zeros((128, 24), f32)
    conv_r[:, 0:8] = r1[CMAP].T
    conv_r[:, 8:16] = r2[CMAP].T
    conv_r[:, 16:24] = uv[CMAP].T

    w1x1T = np.zeros((128, 8, 1024), NP_F16)
    for t in range(8):
        wt = w1x1[:, CMAP[t]] * uv[None, CMAP[t]]  # [1024(o), 128(c)] with taps folded
        for u in range(8):
            w1x1T[:, t, u * 128:(u + 1) * 128] = wt[OMAP[u], :].T.astype(NP_F16)

    inv = 1.0 / np.sqrt(bn_v + BN_EPS)
    bnsc = (inv * bn_g).astype(f32)
    bnbi = (bn_b - bn_m * bnsc).astype(f32)
    bn_sb = np.zeros((128, 16), f32)
    bn_sb[:, 0:8] = bnsc[OMAP].T
    bn_sb[:, 8:16] = bnbi[OMAP].T

    wout = np.concatenate([w_out[h * DH:(h + 1) * DH, :] for h in heads], axis=0).astype(NP_F16)

    ident = np.eye(128, dtype=f32)
    ones_row = np.ones((1, 1024), NP_F16)

    return {
        "x_bp": x,                       # [1024, 256] f32
        "wqkv_a": np.ascontiguousarray(wqkv[0:128]),
        "wqkv_b": np.ascontiguousarray(wqkv[128:256]),
        "wqkv_c": np.ascontiguousarray(wqkv[256:257]),
        "w1x1t": np.ascontiguousarray(w1x1T.reshape(128, 8192)),
        "conv_r": conv_r,
        "bn_sb": bn_sb,
        "wout": wout,
        "ident": ident,
        "ones_row": ones_row,
    }


def gather(parts, b_out):
    out = np.zeros((B, P, N, DIM), np.float32)
    for bp in range(4):
        out[bp // 2, bp % 2] = parts[2 * bp] + parts[2 * bp + 1] + b_out[None, :].astype(np.float32)
    return out


# ---------------------------------------------------------------- device

def declare_tensors(nc):
    f16, f32 = dt.float16, dt.float32
    t = {}
    def mk(name, shape, dty, kind="ExternalInput"):
        t[name] = nc.dram_tensor(name, shape, dty, kind=kind).ap()
    mk("x_bp", [1024, 256], f32)
    mk("wqkv_a", [128, 772], f16)
    mk("wqkv_b", [128, 772], f16)
    mk("wqkv_c", [1, 772], f16)
    mk("w1x1t", [128, 8192], f16)
    mk("conv_r", [128, 24], f32)
    mk("bn_sb", [128, 16], f32)
    mk("wout", [256, 256], f16)
    mk("ident", [128, 128], f32)
    mk("ones_row", [1, 1024], f16)
    mk("out_part", [1024, 256], f32, kind="ExternalOutput")
    return t


def en_wh(ap1024, lo, hi):
    """Enumerate a 1024-wide free range as (outer idx in [lo,hi), inner 32) with
    addr = inner*32 + outer."""
    r = ap1024.rearrange("p (a b) -> p a b", a=32)   # a = addr//32, b = addr%32
    return r.transpose([0, 2, 1])[:, lo:hi, :]


def emit(tc, dr, sim_safe=False, gp_conv=False):
    nc = tc.nc
    f32, f16 = dt.float32, dt.float16

    from contextlib import ExitStack
    stack = ExitStack()
    const = stack.enter_context(tc.tile_pool(name="const", bufs=1))
    xglob = const.tile([128, 8, 256], f32, tag="xglob")
    nc.sync.dma_start(xglob[:], dr["x_bp"].rearrange("(t p) d -> p t d", p=128))
    conv_r = const.tile([128, 24], f32, tag="conv_r")
    nc.sync.dma_start(conv_r[:], dr["conv_r"])
    bn_sb = const.tile([128, 16], f32, tag="bn_sb")
    nc.sync.dma_start(bn_sb[:], dr["bn_sb"])
    wout = const.tile([64, 4, 256], f16, tag="wout")
    for hi in range(4):
        nc.sync.dma_start(wout[:, hi, :], dr["wout"][hi * 64:(hi + 1) * 64])
    ones_row = const.tile([1, 1024], f16, tag="ones_row")
    nc.sync.dma_start(ones_row[:], dr["ones_row"])
    w1x1t = const.tile([128, 8192], f16, tag="w1x1t")

    q_sb = const.tile([64, 4, 1024], f16, tag="q_sb")
    k_sb = const.tile([64, 4, 1024], f16, tag="k_sb")
    v_sb = const.tile([128, 8, 260], f16, tag="v_sb")
    xnt = const.tile([128, 2, 1024], f16, tag="xnt")
    avn = const.tile([64, 4, 1024], f16, tag="avn")
    out_sb = const.tile([128, 8, 256], f32, tag="out_sb")

    work = stack.enter_context(tc.tile_pool(name="work", bufs=1))
    dram = stack.enter_context(tc.tile_pool(name="dram", bufs=1, space="DRAM"))
    ps = stack.enter_context(tc.tile_pool(name="ps", bufs=4, space="PSUM"))

    def conv_chain(ps_s, a3, j):
        ap16 = work.tile([128, 1024], f32, tag="ap16", bufs=3)
        nc.vector.transpose(ap16[:], ps_s[:])
        r1s = conv_r[:, j:j+1]
        r2s = conv_r[:, 8+j:9+j]
        a1 = ap16[:].rearrange("p (w2 w1) -> p w2 w1", w2=32)
        t1 = work.tile([128, 1024], f16, tag="t1", bufs=2)
        t1r = t1[:].rearrange("p (w2 w1) -> p w2 w1", w2=32)
        nc.vector.scalar_tensor_tensor(t1r[:, 0:31, :], a1[:, 1:32, :], r1s, a1[:, 0:31, :],
                                       op0=ALU.mult, op1=ALU.add)
        nc.vector.scalar_tensor_tensor(t1r[:, 31, :], a1[:, 30, :], r1s, a1[:, 31, :],
                                       op0=ALU.mult, op1=ALU.add)
        a3n = a3[:, j, :].rearrange("p (w1 w2) -> p w1 w2", w1=32)
        t1p = t1[:].rearrange("p (w2 w1) -> p w2 w1", w2=32).transpose([0, 2, 1])  # [p, w1, w2]
        nc.vector.scalar_tensor_tensor(a3n[:, 0:31, :], t1p[:, 1:32, :], r2s, t1p[:, 0:31, :],
                                       op0=ALU.mult, op1=ALU.add)
        nc.vector.scalar_tensor_tensor(a3n[:, 31, :], t1p[:, 30, :], r2s, t1p[:, 31, :],
                                       op0=ALU.mult, op1=ALU.add)

    def qk_phase(hi):
        a3 = work.tile([128, 8, 1024], f16, tag="a3", bufs=2, name=f"a3_{hi}")
        for j in range(NT):
            ps_s = ps.tile([128, 1024], f32, tag="ps", bufs=4, name=f"pss{hi}_{j}")
            for ch in range(2):
                sl = slice(ch * 512, (ch + 1) * 512)
                nc.tensor.matmul(ps_s[:, sl], q_sb[:, hi, j * 128:(j + 1) * 128],
                                 k_sb[:, hi, sl], start=True, stop=True)
            conv_chain(ps_s, a3, j)
        return a3

    a3_list = {}

    # ---------------- startup: LN + xnT + qkv ----------------
    with tc.tile_pool(name="startup", bufs=1) as su:
        ident = su.tile([128, 128], f32, tag="ident")
        nc.sync.dma_start(ident[:], dr["ident"])
        wqa = su.tile([128, 772], f16, tag="wqa")
        nc.sync.dma_start(wqa[:], dr["wqkv_a"])
        wqb = su.tile([128, 772], f16, tag="wqb")
        nc.sync.dma_start(wqb[:], dr["wqkv_b"])
        wqc = su.tile([1, 772], f16, tag="wqc")
        nc.sync.dma_start(wqc[:], dr["wqkv_c"])

        x = xglob
        nc.sync.dma_start(w1x1t[:], dr["w1x1t"])
        s1 = su.tile([128, 8], f32, tag="s1")
        s2 = su.tile([128, 8], f32, tag="s2")
        scr = su.tile([128, 256], f32, tag="scr")
        mu8 = su.tile([128, 8], f32, tag="mu8")
        var8 = su.tile([128, 8], f32, tag="var8")
        rstd8 = su.tile([128, 8], f32, tag="rstd8")
        t8 = su.tile([128, 8], f32, tag="t8")
        xn = su.tile([128, 8, 256], f32, tag="xn")
        for j in range(NT):
            nc.vector.tensor_reduce(s1[:, j:j+1], x[:, j, :], mybir.AxisListType.X, ALU.add)
            nc.scalar.activation(scr[:], x[:, j, :], AF.Square, accum_out=s2[:, j:j+1])
        nc.vector.tensor_scalar(mu8[:], s1[:], 1.0 / 256, None, ALU.mult)
        nc.vector.tensor_tensor(t8[:], mu8[:], mu8[:], op=ALU.mult)
        nc.vector.tensor_scalar(t8[:], t8[:], -1.0, LN_EPS, ALU.mult, ALU.add)
        nc.vector.scalar_tensor_tensor(var8[:], s2[:], 1.0 / 256, t8[:], op0=ALU.mult, op1=ALU.add)
        nc.scalar.activation(t8[:], var8[:], AF.Ln)
        nc.scalar.activation(rstd8[:], t8[:], AF.Exp, scale=-0.5)
        for j in range(NT):
            nc.vector.tensor_scalar(xn[:, j, :], x[:, j, :], mu8[:, j:j+1], rstd8[:, j:j+1],
                                    ALU.subtract, ALU.mult)
        for j in range(NT):
            for half in range(2):
                pt = ps.tile([128, 128], f32, tag="ps", bufs=4)
                nc.tensor.transpose(pt[:], xn[:, j, half * 128:(half + 1) * 128], ident[:])
                nc.scalar.activation(xnt[:, half, j * 128:(j + 1) * 128], pt[:], AF.Copy)

        for qk in range(2):  # 0=q, 1=k
            for h in range(4):
                base = qk * 256
                lhs_a = wqa[:, base + h * 64: base + (h + 1) * 64]
                lhs_b = wqb[:, base + h * 64: base + (h + 1) * 64]
                lhs_c = wqc[:, base + h * 64: base + (h + 1) * 64]
                ps_qk = ps.tile([64, 1024], f32, tag="ps", bufs=4)
                for ch in range(2):
                    sl = slice(ch * 512, (ch + 1) * 512)
                    nc.tensor.matmul(ps_qk[:, sl], lhs_a, xnt[:, 0, sl], start=True, stop=False)
                    nc.tensor.matmul(ps_qk[:, sl], lhs_b, xnt[:, 1, sl], start=False, stop=False)
                    nc.tensor.matmul(ps_qk[:, sl], lhs_c, ones_row[:, sl], start=False, stop=True)
                if qk == 0:
                    nc.scalar.activation(q_sb[:, h, :], ps_qk[:], AF.Copy)
                else:
                    # read-permute psum (h2,w2) in (w2,h2) order; write k_sb contiguous
                    kin = ps_qk[:].rearrange("p (h2 w2) -> p h2 w2", h2=32).transpose([0, 2, 1])
                    nc.scalar.activation(k_sb[:, h, :], kin, AF.Copy)
        emit_qk_proj(0, 0)
        emit_qk_proj(0, 1)
        a3_list[0] = qk_phase(0)
        for h, qk in [(1, 0), (1, 1), (2, 0), (2, 1), (3, 0), (3, 1)]:
            emit_qk_proj(h, qk)
        for r in range(NT):
            ps_v = ps.tile([128, 260], f32, tag="ps", bufs=4)
            nsl = slice(r * 128, (r + 1) * 128)
            nc.tensor.matmul(ps_v[:], xnt[:, 0, nsl], wqa[:, 512:772], start=True, stop=False)
            nc.tensor.matmul(ps_v[:], xnt[:, 1, nsl], wqb[:, 512:772], start=False, stop=False)
            nc.tensor.matmul(ps_v[:], ones_row[:, nsl], wqc[:, 512:772], start=False, stop=True)
            nc.scalar.activation(v_sb[:, r, :], ps_v[:], AF.Copy)

    # odd heads' q/k moved to partitions 64..127 for QK row-group packing
    qk_hi = const.tile([128, 2, 2048], f16, tag="qk_hi")
    for p2 in range(2):
        nc.sync.dma_start(qk_hi[64:128, p2, 0:1024], q_sb[0:64, 2 * p2 + 1, :])
        nc.sync.dma_start(qk_hi[64:128, p2, 1024:2048], k_sb[0:64, 2 * p2 + 1, :])

    # ---------------- per-head pipeline ----------------
    work = stack.enter_context(tc.tile_pool(name="work", bufs=1))
    dram = stack.enter_context(tc.tile_pool(name="dram", bufs=1, space="DRAM"))
    ps = stack.enter_context(tc.tile_pool(name="ps", bufs=4, space="PSUM"))

    def conv_chain(ps_s, a3, j):
        ap16 = work.tile([128, 1024], f32, tag="ap16", bufs=3)
        nc.vector.transpose(ap16[:], ps_s[:])
        r1s = conv_r[:, j:j+1]
        r2s = conv_r[:, 8+j:9+j]
        a1 = ap16[:].rearrange("p (w2 w1) -> p w2 w1", w2=32)
        t1 = work.tile([128, 1024], f16, tag="t1", bufs=2)
        t1r = t1[:].rearrange("p (w2 w1) -> p w2 w1", w2=32)
        nc.vector.scalar_tensor_tensor(t1r[:, 0:31, :], a1[:, 1:32, :], r1s, a1[:, 0:31, :],
                                       op0=ALU.mult, op1=ALU.add)
        nc.vector.scalar_tensor_tensor(t1r[:, 31, :], a1[:, 30, :], r1s, a1[:, 31, :],
                                       op0=ALU.mult, op1=ALU.add)
        a3n = a3[:, j, :].rearrange("p (w1 w2) -> p w1 w2", w1=32)
        t1p = t1[:].rearrange("p (w2 w1) -> p w2 w1", w2=32).transpose([0, 2, 1])  # [p, w1, w2]
        nc.vector.scalar_tensor_tensor(a3n[:, 0:31, :], t1p[:, 1:32, :], r2s, t1p[:, 0:31, :],
                                       op0=ALU.mult, op1=ALU.add)
        nc.vector.scalar_tensor_tensor(a3n[:, 31, :], t1p[:, 30, :], r2s, t1p[:, 31, :],
                                       op0=ALU.mult, op1=ALU.add)

    def qk_pair(p2):
        he, ho = 2 * p2, 2 * p2 + 1
        a3e = work.tile([128, 8, 1024], f16, tag="a3", bufs=2, name=f"a3e{p2}")
        a3o = work.tile([128, 8, 1024], f16, tag="a3", bufs=2, name=f"a3o{p2}")
        for j in range(NT):
            ps_se = ps.tile([128, 1024], f32, tag="ps", bufs=4, name=f"pse{p2}_{j}")
            ps_so = ps.tile([128, 1024], f32, tag="ps", bufs=4, name=f"pso{p2}_{j}")
            for ch in range(2):
                sl = slice(ch * 512, (ch + 1) * 512)
                nc.tensor.matmul(ps_se[:, sl], q_sb[:, he, j * 128:(j + 1) * 128],
                                 k_sb[:, he, sl], start=True, stop=True)
                nc.tensor.matmul(ps_so[:, sl], qk_hi[64:128, p2, j * 128:(j + 1) * 128],
                                 qk_hi[64:128, p2, 1024 + ch * 512:1024 + (ch + 1) * 512],
                                 start=True, stop=True, tile_position=(64, 0))
            conv_chain(ps_se, a3e, j)
            conv_chain(ps_so, a3o, j)
        return {he: a3e, ho: a3o}

    a3_list = qk_pair(0)
    for hi in range(4):
        a3 = a3_list.pop(hi)
        # 1x1 conv + BN/SiLU + vtrans2
        s2t_slabs = []
        first_silu_inst = None
        last_silu_inst = None
        for u in range(8):
            ps_y = ps.tile([128, 1024], f32, tag="ps", bufs=4)
            for t in range(8):
                lhs = w1x1t[:, t * 1024 + u * 128: t * 1024 + (u + 1) * 128]
                rhs = a3[:, t, :]
                for ch in range(2):
                    nc.tensor.matmul(ps_y[:, ch * 512:(ch + 1) * 512], lhs,
                                     rhs[:, ch * 512:(ch + 1) * 512],
                                     start=(t == 0), stop=(t == 7))
            ys = work.tile([128, 1024], f32, tag="ys", bufs=3)
            if sim_safe:
                zz = work.tile([128, 1024], f32, tag="zz", bufs=1)
                nc.vector.tensor_scalar(zz[:], ps_y[:], bn_sb[:, u:u+1], bn_sb[:, 8+u:9+u],
                                        ALU.mult, ALU.add)
                sg = work.tile([128, 1024], f32, tag="sg", bufs=1)
                si = nc.scalar.activation(sg[:], ps_y[:], AF.Sigmoid, bias=bn_sb[:, 8+u:9+u],
                                          scale=bn_sb[:, u:u+1])
                nc.vector.tensor_tensor(ys[:], zz[:], sg[:], op=ALU.mult)
            else:
                si = nc.scalar.activation(ys[:], ps_y[:], AF.Silu, bias=bn_sb[:, 8+u:9+u],
                                          scale=bn_sb[:, u:u+1])
            if first_silu_inst is None:
                first_silu_inst = si
            last_silu_inst = si
            s2t_u = work.tile([128, 1024], f32, tag="s2t", bufs=8)
            nc.vector.transpose(s2t_u[:], ys[:])
            s2t_slabs.append(s2t_u)



        if hi == 1:
            a3_list.update(qk_pair(1))
        # exp + AV
        ps_av = ps.tile([128, 1024], f32, tag="ps", bufs=4)  # rows 0:65 used
        first_exp_inst = None
        for r in range(8):
            e = work.tile([128, 1024], f16, tag="e", bufs=3)
            sin = s2t_slabs[r][:].rearrange("p (w1 o1) -> p w1 o1", w1=32).transpose([0, 2, 1])
            ei = nc.scalar.activation(e[:], sin, AF.Exp)
            if first_exp_inst is None:
                first_exp_inst = ei
            lhs = v_sb[:, r, hi * 65: hi * 65 + 65]
            for ch in range(2):
                nc.tensor.matmul(ps_av[0:65, ch * 512:(ch + 1) * 512], lhs,
                                 e[:, ch * 512:(ch + 1) * 512],
                                 start=(r == 0), stop=(r == 7))

        rln = work.tile([1, 1024], f32, tag="rln")
        nc.scalar.activation(rln[:], ps_av[64:65, :], AF.Ln)
        rinv = work.tile([1, 1024], f32, tag="rinv")
        prev_rinv_inst = nc.scalar.activation(rinv[:], rln[:], AF.Exp, scale=-1.0)
        rbc = work.tile([64, 1024], f32, tag="rbc", bufs=2)
        rdram = dram.tile([1, 1024], f32, tag="rdram", bufs=2)
        nc.sync.dma_start(rdram[:], rinv[:])
        rinv_bc = bass.AP(tensor=rdram.tensor, offset=rdram[:].offset,
                          ap=[[0, 64]] + list(rdram[:].ap)[1:])
        nc.sync.dma_start(rbc[:], rinv_bc)
        nc.vector.tensor_tensor(avn[:, hi, :], ps_av[0:64, :], rbc[:], op=ALU.mult)

    # ---------------- out projection ----------------
    for j in range(NT):
        ps_o = ps.tile([128, 1024], f32, tag="ps", bufs=4)  # use [128, 0:256]
        for hi in range(4):
            nc.tensor.matmul(ps_o[:, 0:256], avn[:, hi, j * 128:(j + 1) * 128],
                             wout[:, hi, :], start=(hi == 0), stop=(hi == 3))
        nc.scalar.activation(out_sb[:, j, :], ps_o[:, 0:256], AF.Copy)
    nc.sync.dma_start(dr["out_part"].rearrange("(t p) d -> p t d", p=128), out_sb[:])

    stack.close()


# ================================================================ driver

_CACHE = {}


def _build_program():
    nc = bacc.Bacc("TRN2", target_bir_lowering=False, debug=False, num_devices=8)
    dr = declare_tensors(nc)
    with tile.TileContext(nc) as tc:
        emit(tc, dr, sim_safe=False)
    nc.compile()
    return nc


def kernel(**inputs):
    """Full-input AttentionQV0 forward on 8 TRN2 NeuronCores.

    Shards (b,p,head-group) across cores, runs the Bass kernel, gathers
    partial head-group outputs host-side (sum core pairs + b_out).
    """
    from concourse.bass_utils import run_bass_kernel_spmd
    inputs = {k: np.asarray(v) for k, v in inputs.items()}
    if "nc" not in _CACHE:
        _CACHE["nc"] = _build_program()
    nc = _CACHE["nc"]
    in_maps = [host_prep(c, inputs) for c in range(8)]
    res = run_bass_kernel_spmd(nc, in_maps, list(range(8)))
    parts = [res.results[c]["out_part"] for c in range(8)]
    return gather(parts, inputs["b_out"])
